# revision 19
# baseline (speedup 1.0000x reference)
"""Trainium2 Bass kernel for nn_Attention_12034498363898.

Per batch b (B=8 batches, one NeuronCore each):
  xs = x[::2,::2,::2]                    (4096, 64)
  f = xs@Wf+bf; g = xs@Wg+bg
  s = g @ f^T (4096,4096); e = exp(s)
  hv = xs@Wh@gWv + (bh@gWv + g*bv)  (value path pre-projected: (beta@h)@Wv
                                     == beta@(h@Wv) by associativity)
  o = e @ [hv, 1];  v = o[:, :64] / o[:, 64]   (sumexp via ones column)
  out = Up2x(v) + x                      (gamma folded into Wv/bv host-side)

Implementation notes:
  - Data-parallel over batch: 8 cores, one batch each, no collectives.
  - s computed TRANSPOSED per 512-column block (keys m on partitions) with
    fp8(e4m3) f/g operands in DoubleRow perf mode.  The second k-tile plane
    is zeros (c=8 needs no split); layout [8, (f z g z), 4096] makes both
    (f, z) and (g, z) 3-D slices plain APs.
  - o accumulated NATURALLY (queries n on partitions): eT chunks [128m,128n]
    are the stationary operand, hv_aug [128m, 65] streams -> 65 cols/chunk.
  - exp split across ScalarE (exact), VectorE + GpSimd (Schraudolph bit
    trick into bf16 bits, ~2-3% rel err) to balance engine load.
  - Residual interleaved per n-block: x tile prefetched, v bounced through
    a bf16 DRAM scratch to become per-partition contiguous, adds split
    DVE/GpSimd, out written per block.  No serial residual tail.
  - Residual partition layout p=(hb, wb, h2l, w2) so the 4x duplicated
    v-block reads are contiguous-partition DMAs.
"""

import numpy as np

import concourse.bass as bass
import concourse.mybir as mybir
import concourse.tile as tile
from concourse.bass_utils import run_bass_kernel_spmd
from concourse.vector_clock import ScopedClock

# ---------------------------------------------------------------------------
# Workaround: this neuronxcc/walrus build rejects instructions with more than
# one sync-wait command ("Too many sync wait commands" in setupSyncWait).
# (a) TileContext's exit drain carries every outstanding wait -> split into a
#     chain of 1-wait drains.
# (b) Body instructions can get multiple waits from the scheduler -> move
#     extras onto Drain carriers inserted just before, same engine.
_MAX_WAITS = 1


def _split_drain_and_barrier(self, tick_clock, wait_clock):
    import bass_rust

    drain_inst = self.nc.sync.drain()
    wait_clock.add_sem_waits(
        drain_inst.ins, ScopedClock({None: tick_clock.global_clock})
    )
    si = drain_inst.ins.sync_info
    waits = list(si.on_wait)
    if len(waits) > _MAX_WAITS:
        si.on_wait = waits[:_MAX_WAITS]
        drain_inst.ins.sync_info = si
        for k in range(_MAX_WAITS, len(waits), _MAX_WAITS):
            extra = self.nc.sync.drain()
            esi = extra.ins.sync_info
            if esi is None:
                esi = bass_rust.SyncInfo(
                    on_wait=waits[k : k + _MAX_WAITS], on_update=[]
                )
            else:
                esi.on_wait = waits[k : k + _MAX_WAITS]
            extra.ins.sync_info = esi

    self.nc.all_engine_barrier()
    assert self.sems is not None
    popped = self.nc._tile_sem_poison_stack.pop()
    assert popped is self._sem_poison
    self.nc.clear_and_free_semaphores(list(self.sems.allocated().values()))
    self.nc.all_engine_barrier()


tile.TileContext._drain_and_barrier = _split_drain_and_barrier

_orig_lower_ordered = tile.TileContext._lower_ordered_insts


def _split_waits_lower(self, ordered):
    import bass_rust

    for bb, insts in ordered.items():
        new = []
        for inst in insts:
            si = getattr(inst, "sync_info", None)
            waits = list(si.on_wait) if si is not None else []
            if len(waits) > _MAX_WAITS:
                eng = inst.engine
                for w in waits[:-_MAX_WAITS]:
                    carrier = self.nc.engines[eng].drain(fusable=False).ins
                    csi = carrier.sync_info
                    if csi is None:
                        csi = bass_rust.SyncInfo(on_wait=[w], on_update=[])
                    else:
                        csi.on_wait = [w]
                    carrier.sync_info = csi
                    new.append(carrier)
                si.on_wait = waits[-_MAX_WAITS:]
                inst.sync_info = si
            new.append(inst)
        insts[:] = new
    return _orig_lower_ordered(self, ordered)


tile.TileContext._lower_ordered_insts = _split_waits_lower
# ---------------------------------------------------------------------------

F32 = mybir.dt.float32
I16 = mybir.dt.int16
BF16 = mybir.dt.bfloat16
FP8 = mybir.dt.float8e4

B = 8
HH = 32
N = 4096          # subsampled positions per batch
C = 64
NROWS = 32768     # full-res rows per batch
NB = 8            # n-blocks of 512
MC = 32           # m-chunks of 128

# Schraudolph fast-exp (bf16-bits variant): exp(x) ~= bf16_bits(x*A + Bc)
SCH_A = 184.6650      # 2^7 / ln 2
SCH_B = 16248.58      # 127*2^7 - 486411/2^16

# Per-nb exp engine schedule: 32 tiles of [128,512] across 3 engines.
EXP_SCHED = "ADPADAPADPADAPADPADAPADPADAPADPA"  # 14 A, 9 D, 9 P
# xsT copy engine per mc (mod 8)
XST_SCHED = "ADAPDADP"
USE_DR = True         # fp8 DoubleRow for the s matmul
POOL_EXP = True       # allow gpsimd Schraudolph tiles
RESID_SPLIT = 9       # d2 blocks (of 16) on DVE; rest on gpsimd


def build_kernel(use_dr=USE_DR, pool_exp=POOL_EXP, exp_sched=EXP_SCHED,
                 resid_split=RESID_SPLIT, phases=99):
    nc = bass.Bass()

    x = nc.declare_dram_parameter("x", [NROWS, C], F32, isOutput=False)
    wf = nc.declare_dram_parameter("wf", [65, 8], BF16, isOutput=False)
    wg = nc.declare_dram_parameter("wg", [65, 8], BF16, isOutput=False)
    whv = nc.declare_dram_parameter("whv", [65, 65], BF16, isOutput=False)
    ident = nc.declare_dram_parameter("ident", [128, 128], F32, isOutput=False)
    zz = nc.declare_dram_parameter("zz", [8, N], FP8, isOutput=False)
    out = nc.declare_dram_parameter("out", [NROWS, C], F32, isOutput=True)

    vscratch = nc.dram_tensor("vscratch", [N, C], BF16)
    # write view: v chunk ch rows [128ch, 128ch+128), partition j = row
    vsc_w = vscratch.rearrange("(ch p) c -> p ch c", p=128)
    # read view: 16-row v blocks, one per partition
    vsc_r = vscratch.rearrange("(vb rr) c -> vb (rr c)", rr=16)

    # subsampled rows: r = h2*2048 + hb*1024 + w2*64 + wb*32 + d2*2 + db
    x_sub = x.rearrange(
        "(h2 hb w2 wb d2 db) c -> hb wb db h2 w2 d2 c",
        h2=16, hb=2, w2=16, wb=2, d2=16, db=2,
    )[0, 0, 0]  # [16 h2, 16 w2, 16 d2, 64] subsampled rows

    def xs_chunk(mc):  # [8, 16, 64] -> 128 rows of xs
        return x_sub[mc >> 1, 8 * (mc & 1) : 8 * (mc & 1) + 8]

    # residual views: partition p = (h2l, hb, w2, wb) = natural row order,
    # so x/out tiles are contiguous row slices.
    x_res = x.rearrange("(b p f) c -> b p (f c)", b=NB, p=128)
    out_res = out.rearrange("(b p f) c -> b p (f c)", b=NB, p=128)

    def vblk_dup_view(t):  # [128, 1024] tile -> [hb, wb, 2 h2l, 16 w2, f]
        return t[:].rearrange(
            "(h2l hb w2 wb) f -> hb wb h2l w2 f", hb=2, wb=2, h2l=2, w2=16
        )

    with tile.TileContext(nc) as tc:
        with (
            tc.tile_pool(name="const", bufs=1) as const_pool,
            tc.tile_pool(name="persist", bufs=1) as persist,
            tc.tile_pool(name="stage", bufs=8) as stage,
            tc.tile_pool(name="et", bufs=10) as et_pool,
            tc.tile_pool(name="vsb", bufs=2) as vsb_pool,
            tc.tile_pool(name="vblk", bufs=2) as vblk_pool,
            tc.tile_pool(name="xres", bufs=2) as xres_pool,
            tc.tile_pool(name="ores", bufs=2) as ores_pool,
            tc.tile_pool(name="p2", bufs=6, space=bass.MemorySpace.PSUM) as p2,
            tc.tile_pool(name="po", bufs=2, space=bass.MemorySpace.PSUM) as po,
        ):
            # ---- constants ----
            id_sb = const_pool.tile([128, 128], F32)
            nc.sync.dma_start(id_sb[:], ident[:])
            wf_sb = const_pool.tile([65, 8], BF16)
            nc.sync.dma_start(wf_sb[:], wf[:])
            wg_sb = const_pool.tile([65, 8], BF16)
            nc.sync.dma_start(wg_sb[:], wg[:])
            whv_sb = const_pool.tile([65, 65], BF16)
            nc.sync.dma_start(whv_sb[:], whv[:])

            # f/g fp8 planes: [8, (f, zeros, g, zeros), N]
            sdt = FP8 if use_dr else BF16
            fgz_flat = persist.tile([8, 4 * N], sdt)
            fgz = fgz_flat[:].rearrange("p (pl n) -> p pl n", pl=4)
            if use_dr:
                nc.sync.dma_start(fgz[:, 1, :], zz[:])
                nc.sync.dma_start(fgz[:, 3, :], zz[:])

            # ---- phase 1: xs load + transpose + projections, pipelined ----
            # Software-pipelined: the hv matmul for chunk mc-2 is issued
            # after the transpose of chunk mc, so PE never waits on the
            # xsT copies; f/g projections similarly lag one 512-slice.
            xsT = persist.tile([65, N], BF16)
            nc.vector.memset(xsT[64:65, :], 1.0)
            hv_aug = persist.tile([128, MC * 65], BF16)
            cp_eng = {"A": nc.scalar, "D": nc.vector, "P": nc.gpsimd}

            def hv_proj(mc):
                ph = p2.tile([128, 65], F32, tag="sT")
                nc.tensor.matmul(
                    ph[:], xsT[:, mc * 128 : (mc + 1) * 128], whv_sb[:],
                    start=True, stop=True,
                )
                nc.gpsimd.tensor_copy(
                    hv_aug[:, mc * 65 : (mc + 1) * 65], ph[:]
                )

            def fg_proj(nbs):
                pf = p2.tile([8, 512], F32, tag="sT")
                nc.tensor.matmul(
                    pf[:], wf_sb[:], xsT[:, nbs * 512 : (nbs + 1) * 512],
                    start=True, stop=True,
                )
                nc.scalar.copy(fgz[:, 0, nbs * 512 : (nbs + 1) * 512], pf[:])
                pg = p2.tile([8, 512], F32, tag="sT")
                nc.tensor.matmul(
                    pg[:], wg_sb[:], xsT[:, nbs * 512 : (nbs + 1) * 512],
                    start=True, stop=True,
                )
                nc.vector.tensor_copy(
                    fgz[:, 2, nbs * 512 : (nbs + 1) * 512], pg[:]
                )

            for mc in range(MC):
                st = stage.tile([128, C], F32, tag="xs_stage")
                nc.sync.dma_start(st[:], xs_chunk(mc))
                tp = p2.tile([64, 128], F32, tag="sT")
                nc.tensor.transpose(tp[:], st[:], id_sb[:])
                eng = cp_eng[XST_SCHED[mc % 8]]
                if eng is nc.scalar:
                    eng.copy(xsT[0:64, mc * 128 : (mc + 1) * 128], tp[:])
                else:
                    eng.tensor_copy(
                        xsT[0:64, mc * 128 : (mc + 1) * 128], tp[:]
                    )
                if mc >= 2:
                    hv_proj(mc - 2)
                if mc % 4 == 1 and mc >= 4:
                    fg_proj(mc // 4 - 1)
            hv_proj(MC - 2)
            hv_proj(MC - 1)
            fg_proj(MC // 4 - 1)

            if phases < 3:
                return nc

            # ---- attention + residual, per n-block of 512 ----
            # The residual pipeline for block nb-1 is threaded through block
            # nb's instruction stream at staggered points so that every DMA's
            # waits are already satisfied when the in-order SP queue reaches
            # it (an unmet wait head-blocks all later DMAs on the queue).
            LOOK = 4            # o-accum lags s/exp by this many m-chunks
            K_VW, K_VR, K_RES, K_OUT = 2, 12, 20, 28
            PRIO_OFF = 300

            state = {}  # nb -> dict with tiles for the trailing residual

            def stage_vw(nb):  # v(nb) -> DRAM scratch
                v_sb = state[nb]["v_sb"]
                for half in range(2):
                    nc.sync.dma_start(
                        vsc_w[:, 4 * nb + 2 * half : 4 * nb + 2 * half + 2, :],
                        v_sb[:, 2 * half : 2 * half + 2, :],
                    )

            def stage_vr(nb):  # v blocks (4x duplicated across (hb, wb))
                vbt = vblk_pool.tile([128, 1024], BF16, tag="vblk")
                state[nb]["vbt"] = vbt
                vbt_dup = vblk_dup_view(vbt)
                for hb in range(2):
                    for wb in range(2):
                        nc.sync.dma_start(
                            vbt_dup[hb, wb],
                            vsc_r[32 * nb : 32 * (nb + 1)].rearrange(
                                "(h2l w2) f -> h2l w2 f", h2l=2
                            ),
                        )

            def stage_resid(nb):
                xt, vbt = state[nb]["xt"], state[nb]["vbt"]
                ot = ores_pool.tile([128, 2048], F32, tag="ot")
                state[nb]["ot"] = ot
                v_b = vbt[:].rearrange(
                    "p (d2 uu c) -> p d2 uu c", d2=16, uu=1
                ).broadcast_to([128, 16, 2, C])
                sp = resid_split
                nc.vector.tensor_tensor(
                    ot[:, : sp * 128].rearrange(
                        "p (d2 db c) -> p d2 db c", db=2, c=C
                    ),
                    xt[:, : sp * 128].rearrange(
                        "p (d2 db c) -> p d2 db c", db=2, c=C
                    ),
                    v_b[:, :sp],
                    mybir.AluOpType.add,
                )
                nc.gpsimd.tensor_tensor(
                    ot[:, sp * 128 :].rearrange(
                        "p (d2 db c) -> p d2 db c", db=2, c=C
                    ),
                    xt[:, sp * 128 :].rearrange(
                        "p (d2 db c) -> p d2 db c", db=2, c=C
                    ),
                    v_b[:, sp:],
                    mybir.AluOpType.add,
                )

            def stage_out(nb):
                nc.sync.dma_start(out_res[nb], state.pop(nb)["ot"])

            for nb in range(NB):
                state[nb] = {}
                o_ps = po.tile([128, 4, 65], F32, tag="o")
                eTs = {}

                def o_accum(mc):
                    eT = eTs.pop(mc)
                    for q in range(4):
                        nc.tensor.matmul(
                            o_ps[:, q, :],
                            eT[:, q * 128 : (q + 1) * 128],
                            hv_aug[:, mc * 65 : (mc + 1) * 65],
                            start=(mc == 0), stop=(mc == MC - 1),
                            skip_group_check=True,
                        )

                for mc in range(MC):
                    if mc == K_VW:
                        with tc.high_priority(offset=PRIO_OFF):
                            xt = xres_pool.tile([128, 2048], F32, tag="xt")
                            state[nb]["xt"] = xt
                            nc.sync.dma_start(xt[:], x_res[nb])
                            if nb > 0:
                                stage_vw(nb - 1)
                    if mc == K_VR and nb > 0:
                        with tc.high_priority(offset=PRIO_OFF):
                            stage_vr(nb - 1)
                    if mc == K_RES and nb > 0 and phases >= 4:
                        with tc.high_priority(offset=PRIO_OFF):
                            stage_resid(nb - 1)
                    if mc == K_OUT and nb > 0 and phases >= 4:
                        with tc.high_priority(offset=PRIO_OFF):
                            stage_out(nb - 1)
                    sT = p2.tile([128, 512], F32, tag="sT")
                    if use_dr:
                        nc.tensor.matmul(
                            sT[:],
                            fgz[:, 0:2, mc * 128 : (mc + 1) * 128],
                            fgz[:, 2:4, nb * 512 : (nb + 1) * 512],
                            start=True, stop=True,
                            perf_mode=mybir.MatmulPerfMode.DoubleRow,
                        )
                    else:
                        nc.tensor.matmul(
                            sT[:],
                            fgz[:, 0, mc * 128 : (mc + 1) * 128],
                            fgz[:, 2, nb * 512 : (nb + 1) * 512],
                            start=True, stop=True,
                        )
                    eT = et_pool.tile([128, 512], BF16, tag="eT")
                    eTs[mc] = eT
                    e = exp_sched[mc]
                    if e == "P" and not pool_exp:
                        e = "D"
                    if e == "A":
                        nc.scalar.activation(
                            eT[:], sT[:], mybir.ActivationFunctionType.Exp,
                        )
                    elif e == "D":
                        nc.vector.tensor_scalar(
                            eT[:].bitcast(I16), sT[:], SCH_A, SCH_B,
                            mybir.AluOpType.mult, mybir.AluOpType.add,
                        )
                    else:
                        nc.gpsimd.tensor_scalar(
                            eT[:].bitcast(I16), sT[:], SCH_A, SCH_B,
                            mybir.AluOpType.mult, mybir.AluOpType.add,
                        )
                    if mc >= LOOK:
                        o_accum(mc - LOOK)
                for mc in range(MC - LOOK, MC):
                    o_accum(mc)

                # normalize: v = o[:, :64] / o[:, 64]  (bias already inside)
                inv = vsb_pool.tile([128, 4], F32, tag="inv")
                v_sb = vsb_pool.tile([128, 4, C], BF16, tag="vsb")
                state[nb]["v_sb"] = v_sb
                for q in range(4):
                    nc.vector.reciprocal(
                        inv[:, q : q + 1], o_ps[:, q, 64:65]
                    )
                    nc.vector.tensor_scalar(
                        v_sb[:, q, :], o_ps[:, q, 0:C],
                        inv[:, q : q + 1], None,
                        mybir.AluOpType.mult,
                    )

            # drain the last block's residual
            stage_vw(NB - 1)
            stage_vr(NB - 1)
            if phases >= 4:
                stage_resid(NB - 1)
                stage_out(NB - 1)

    return nc


_CACHE = {}


def _get_nc():
    if "nc" not in _CACHE:
        _CACHE["nc"] = build_kernel()
    return _CACHE["nc"]


def _make_in_maps(inputs):
    import ml_dtypes

    bf16 = ml_dtypes.bfloat16
    fp8 = ml_dtypes.float8_e4m3fn
    x = np.asarray(inputs["x"], dtype=np.float32)
    gamma_v = float(np.asarray(inputs["gamma"]).reshape(-1)[0])
    wf_aug = np.concatenate(
        [np.asarray(inputs["Wf"]), np.asarray(inputs["bf"])[None, :]], 0
    ).astype(np.float32)
    wg_aug = np.concatenate(
        [np.asarray(inputs["Wg"]), np.asarray(inputs["bg"])[None, :]], 0
    ).astype(np.float32)
    # value path pre-projected through gamma*Wv (+ gamma*bv), ones col 64
    wh = np.asarray(inputs["Wh"], np.float32)
    bh = np.asarray(inputs["bh"], np.float32)
    wv = np.asarray(inputs["Wv"], np.float32)
    bv = np.asarray(inputs["bv"], np.float32)
    whv_aug = np.zeros((65, 65), np.float32)
    whv_aug[:64, :64] = wh @ (gamma_v * wv)
    whv_aug[64, :64] = bh @ (gamma_v * wv) + gamma_v * bv
    whv_aug[64, 64] = 1.0
    shared = {
        "wf": wf_aug.astype(bf16),
        "wg": wg_aug.astype(bf16),
        "whv": whv_aug.astype(bf16),
        "ident": np.eye(128, dtype=np.float32),
        "zz": np.zeros((8, N), fp8),
    }
    return [
        dict(shared, x=np.ascontiguousarray(x[b].reshape(NROWS, C)))
        for b in range(B)
    ]


def kernel(x, Wf, bf, Wg, bg, Wh, bh, Wv, bv, gamma):
    nc = _get_nc()
    in_maps = _make_in_maps(dict(
        x=x, Wf=Wf, bf=bf, Wg=Wg, bg=bg, Wh=Wh, bh=bh, Wv=Wv, bv=bv,
        gamma=gamma,
    ))
    res = run_bass_kernel_spmd(nc, in_maps, list(range(B)))
    outs = [res.results[b]["out"].reshape(HH, HH, HH, C) for b in range(B)]
    return np.stack(outs).astype(np.float32)


if __name__ == "__main__":
    import reference

    inputs = {k: np.asarray(v) for k, v in reference.setup_inputs().items()}
    got = kernel(**inputs)
    exp = np.asarray(reference.reference(**inputs))
    err = np.abs(got - exp).max() / (np.abs(exp).max() + 1e-30)
    print("Relative error:", err)
